# revision 15
# baseline (speedup 1.0000x reference)
"""Trainium2 Bass kernel: 2D valid cross-correlation (4096x4096 image, 15x15 kernel).

Strategy: shard output COLUMNS across 8 NeuronCores (spatial data-parallel, no
device-to-device communication). Each core computes the full 4082 output rows
for its 512 output columns.

On-core formulation (space-to-depth / patch matmul): the image is tiled into
8x16 = 128-pixel patches. Host-side, X is repacked so SBUF partition
k = r*16 + c holds within-patch position (r, c), and the free dimension
enumerates patches p = q*33 + s (33 column-patches per patch-row, the 33rd
being horizontal halo). An output patch (8x16 = 128 pixels, the matmul M dim)
draws its 15x15 receptive field from exactly 6 input patches: row offsets dq
in {0,1,2} x col offsets ds in {0,1}. For each (dq, ds) the map from
input-patch pixels to output-patch pixels is a fixed DENSE 128x128 matrix
  S[(rk,ck),(rm,cm)] = w[8*dq + rk - rm, 16*ds + ck - cm]  (0 out of range)
so the conv is 6 PSUM-accumulated bf16 matmuls per 512-patch chunk. A chunk
is 16 patch-rows x 32 real column-patches, addressed with a 3D moving AP
(stride 33 between patch-rows) so the halo column is never streamed or
written back. This streams 6 PE columns per 128 outputs (vs ~15 per 114 for
banded-Toeplitz), uses the full 128x128 array, and bf16 enables the
automatic fast-weight-load path, hiding all stationary reloads.

DMA: each HWDGE queue (sync=SP, scalar=Activation) generates descriptors at
only ~145 GB/s, so input slabs are split across both queues up front (the PE
consumes input at ~200 GB/s), outputs go out staged in 4-chunk groups early
(scalar/gpsimd), and the tail shrinks to singles alternating scalar/sync.

bf16 inputs give ~1.4e-3 relative error vs the fp32 reference (tol 2e-2):
products are exact in the PE, accumulation is fp32 in PSUM.
"""

import numpy as np
import ml_dtypes

import concourse.bass as bass
import concourse.mybir as mybir
import concourse.tile as tile
from concourse import bacc
from concourse.bass_utils import run_bass_kernel_spmd

H, W = 4096, 4096
KH, KW = 15, 15
OH, OW = H - KH + 1, W - KW + 1  # 4082 x 4082

NCORES = 8
COLS_PER_CORE = 512               # output cols per core (core 7: 498 valid)
IN_COLS = COLS_PER_CORE + KW - 1  # 526 input cols (with halo)

PR, PC = 8, 16                    # patch = 8 rows x 16 cols = 128 = PE contraction
SC = 33                           # input col-patches per patch-row (528 cols)
SCO = 32                          # output col-patches per patch-row (512 cols)
QI = 514                          # input patch-rows (4096 rows + 16 pad = 4112)
QO = 511                          # output patch-rows (4082 = 510*8 + 2)
NPI = QI * SC                     # 16962 input patches
NPO = QO * SCO                    # 16352 output patches (no halo col)

RT = 16                           # patch-rows per chunk
NT = RT * SCO                     # 512 output patches = one fp32 PSUM bank
NCHUNKS = 32                      # 31 x 512 + 480 (last chunk: 15 patch-rows)
PASSES = [(0, 0), (0, 1), (1, 0), (1, 1), (2, 0), (2, 1)]

# Input slabs: (first_chunk, last_chunk_excl, queue). Emitted up front, before
# any output DMA, so input never queues behind stores. Fine-grained at the
# head so both queues together outrun the PE from chunk 0.
SLAB_PLAN = [
    (0, 2, "sync"),
    (2, 4, "scalar"),
    (4, 8, "scalar"),
    (8, 16, "sync"),
    (16, 24, "scalar"),
    (24, 32, "sync"),
]
# widest slab: 8 chunks + 2 halo patch-rows (+1: the 3D moving view is sliced
# as nr*SC before dropping the halo column, so it needs one spare patch)
SLABW_MAX = SC * (RT * 8 + 2) + 1

F32 = mybir.dt.float32
BF16 = mybir.dt.bfloat16

# Output DMA plan: (first_chunk, n_chunks, queue). 4-chunk staged groups
# early (scalar + gpsimd), singles alternating sync/scalar at the tail, the
# final chunk split across both HWDGE queues.
OUT_PLAN = [
    (0, 4, "scalar"),
    (4, 4, "gpsimd"),
    (8, 4, "scalar"),
    (12, 4, "gpsimd"),
    (16, 4, "scalar"),
    (20, 2, "sync"),
    (22, 2, "sync"),
    (24, 2, "gpsimd"),
    (26, 2, "scalar"),
    (28, 1, "sync"),
    (29, 1, "scalar"),
    (30, 1, "sync"),
    (31, 1, "split"),
]

N_WARMUP = 27


def _chunk_rows(c):
    """(first_patch_row, n_patch_rows) of chunk c."""
    r0 = RT * c
    return r0, min(RT, QO - r0)


def _build_program():
    nc = bacc.Bacc("TRN2", target_bir_lowering=False, debug=False)
    x = nc.dram_tensor("x", [128, NPI], BF16, kind="ExternalInput").ap()
    wt = nc.dram_tensor("wt", [128, 6 * 128], BF16, kind="ExternalInput").ap()
    out = nc.dram_tensor("out", [128, NPO], F32, kind="ExternalOutput").ap()

    chunk_group = {}
    for g in OUT_PLAN:
        for c in range(g[0], g[0] + g[1]):
            chunk_group[c] = g
    chunk_slab = {}
    for si, (c0, c1, q) in enumerate(SLAB_PLAN):
        for c in range(c0, c1):
            chunk_slab[c] = si

    with tile.TileContext(nc) as tc:
        with (
            tc.tile_pool(name="wpool", bufs=1) as wpool,
            tc.tile_pool(name="xpool", bufs=4) as xpool,
            tc.tile_pool(name="opool", bufs=4) as opool,
            tc.tile_pool(name="dpool", bufs=1) as dpool,
            tc.tile_pool(name="ppool", bufs=7, space="PSUM") as ppool,
            tc.tile_pool(name="dps", bufs=1, space="PSUM") as dps,
        ):
            # All input DMAs up front: weights on scalar, slabs per plan.
            wtile = wpool.tile([128, 6 * 128], BF16, tag="wt")
            nc.scalar.dma_start(wtile[:], wt[:])

            slabs = []
            for c0, c1, q in SLAB_PLAN:
                p_lo = SC * RT * c0
                p_hi = min(SC * (RT * c1 + 2) + 1, NPI)
                st = xpool.tile([128, SLABW_MAX], BF16, tag="xs")
                {"sync": nc.sync, "scalar": nc.scalar}[q].dma_start(
                    st[:, : p_hi - p_lo], x[:, p_lo:p_hi]
                )
                slabs.append((st, p_lo))

            # HAM pre-warm: the PE clock-gate only opens to 2.4GHz after
            # ~3.4us of sustained activity. Dummy matmuls fill the window
            # while slab 0 lands so the real stream starts at full clock.
            dz = dpool.tile([128, 128], F32, tag="dz")
            nc.vector.memset(dz[:], 0)
            dummy = dpool.tile([128, 128], BF16, tag="dummy")
            nc.vector.tensor_copy(dummy[:], dz[:])
            dacc = dps.tile([128, 128], F32)
            for _ in range(N_WARMUP):
                nc.tensor.matmul(dacc[:], dummy[:], dummy[:], start=True, stop=True)

            dma_q = {"sync": nc.sync, "scalar": nc.scalar, "gpsimd": nc.gpsimd}
            stage = None
            for c in range(NCHUNKS):
                st, p_lo = slabs[chunk_slab[c]]
                r0, nr = _chunk_rows(c)
                n = nr * SCO
                acc = ppool.tile([128, NT], F32)
                for i, (dq, ds) in enumerate(PASSES):
                    off = (r0 + dq) * SC + ds - p_lo
                    # moving operand: nr patch-rows of 32 patches, stride 33
                    mv = st[:, off : off + nr * SC].rearrange(
                        "p (r s) -> p r s", s=SC
                    )[:, :, :SCO]
                    nc.tensor.matmul(
                        acc[:, :n],
                        wtile[:, i * 128 : (i + 1) * 128],
                        mv,
                        start=(i == 0),
                        stop=(i == 5),
                    )
                g0, gn_chunks, q = chunk_group[c]
                gi = c - g0
                if gi == 0:
                    stage = opool.tile([128, 4 * NT], F32, tag="ot")
                nc.vector.tensor_copy(stage[:, gi * NT : gi * NT + n], acc[:, :n])
                if gi == gn_chunks - 1:
                    gp0 = NT * g0
                    gn = NT * gi + n
                    if q == "split":
                        h = (gn // 2 + 1) & ~1
                        nc.scalar.dma_start(out[:, gp0 : gp0 + h], stage[:, :h])
                        nc.sync.dma_start(
                            out[:, gp0 + h : gp0 + gn], stage[:, h:gn]
                        )
                    else:
                        dma_q[q].dma_start(out[:, gp0 : gp0 + gn], stage[:, :gn])
    nc.finalize()
    return nc


def _pack_weights(weight: np.ndarray) -> np.ndarray:
    """6 dense 128x128 stationary matrices (one per (dq, ds) pass)."""
    wt = np.zeros((128, 6 * 128), dtype=np.float32)
    rk, ck = np.divmod(np.arange(128)[:, None], PC)  # input pixel within patch
    rm, cm = np.divmod(np.arange(128)[None, :], PC)  # output pixel within patch
    for i, (dq, ds) in enumerate(PASSES):
        a = PR * dq + rk - rm
        b = PC * ds + ck - cm
        valid = (a >= 0) & (a < KH) & (b >= 0) & (b < KW)
        wt[:, i * 128 : (i + 1) * 128] = np.where(
            valid, weight[np.clip(a, 0, KH - 1), np.clip(b, 0, KW - 1)], 0.0
        )
    return wt


def kernel(X: np.ndarray, weight: np.ndarray, bias: np.ndarray) -> np.ndarray:
    X = np.ascontiguousarray(X, dtype=np.float32)
    weight = np.ascontiguousarray(weight, dtype=np.float32)
    bias = np.asarray(bias, dtype=np.float32)

    wt = _pack_weights(weight).astype(ml_dtypes.bfloat16)

    in_maps = []
    for c in range(NCORES):
        xs = np.zeros((QI * PR, SC * PC), dtype=np.float32)
        c0 = c * COLS_PER_CORE
        c1 = min(c0 + IN_COLS, W)
        xs[:H, : c1 - c0] = X[:, c0:c1]
        x_s2d = (
            xs.reshape(QI, PR, SC, PC)
            .transpose(1, 3, 0, 2)
            .reshape(128, NPI)
            .astype(ml_dtypes.bfloat16)
        )
        in_maps.append({"x": np.ascontiguousarray(x_s2d), "wt": wt})

    nc = _build_program()
    res = run_bass_kernel_spmd(nc, in_maps, core_ids=list(range(NCORES)))
    global _last_results
    _last_results = res

    out = np.empty((OH, OW), dtype=np.float32)
    for c in range(NCORES):
        o = np.asarray(res.results[c]["out"], dtype=np.float32)
        o4 = (
            o.reshape(PR, PC, QO, SCO)
            .transpose(2, 0, 3, 1)
            .reshape(QO * PR, SCO * PC)
        )
        c0 = c * COLS_PER_CORE
        n = min(COLS_PER_CORE, OW - c0)
        out[:, c0 : c0 + n] = o4[:OH, :n]

    b0 = float(bias.reshape(-1)[0]) if bias.size else 0.0
    if b0 != 0.0:
        out += b0
    return out


# revision 17
# speedup vs baseline: 1.0743x; 1.0743x over previous
"""Trainium2 Bass kernel: 2D valid cross-correlation (4096x4096 image, 15x15 kernel).

Strategy: shard output COLUMNS across 8 NeuronCores (spatial data-parallel, no
device-to-device communication). Each core computes the full 4082 output rows
for its 512 output columns.

On-core formulation (space-to-depth / patch matmul): the image is tiled into
8x16 = 128-pixel patches. Host-side, X is repacked so SBUF partition
k = r*16 + c holds within-patch position (r, c), and the free dimension
enumerates patches p = q*33 + s (33 column-patches per patch-row, the 33rd
being horizontal halo). An output patch (8x16 = 128 pixels, the matmul M dim)
draws its 15x15 receptive field from exactly 6 input patches: row offsets dq
in {0,1,2} x col offsets ds in {0,1}. For each (dq, ds) the map from
input-patch pixels to output-patch pixels is a fixed DENSE 128x128 matrix
  S[(rk,ck),(rm,cm)] = w[8*dq + rk - rm, 16*ds + ck - cm]  (0 out of range)
so the conv is 6 PSUM-accumulated bf16 matmuls per 512-patch chunk. A chunk
is 16 patch-rows x 32 real column-patches, addressed with a 3D moving AP
(stride 33 between patch-rows) so the halo column is never streamed or
written back. This streams 6 PE columns per 128 outputs (vs ~15 per 114 for
banded-Toeplitz), uses the full 128x128 array, and bf16 enables the
automatic fast-weight-load path, hiding all stationary reloads.

DMA: each HWDGE queue (sync=SP, scalar=Activation) generates descriptors at
only ~145 GB/s, so input slabs are split across both queues up front (the PE
consumes input at ~200 GB/s), outputs go out staged in 4-chunk groups early
(scalar/gpsimd), and the tail shrinks to singles alternating scalar/sync.

bf16 inputs give ~1.4e-3 relative error vs the fp32 reference (tol 2e-2):
products are exact in the PE, accumulation is fp32 in PSUM.
"""

import numpy as np
import ml_dtypes

import concourse.bass as bass
import concourse.mybir as mybir
import concourse.tile as tile
from concourse import bacc
from concourse.bass_utils import run_bass_kernel_spmd

H, W = 4096, 4096
KH, KW = 15, 15
OH, OW = H - KH + 1, W - KW + 1  # 4082 x 4082

NCORES = 8
COLS_PER_CORE = 512               # output cols per core (core 7: 498 valid)
IN_COLS = COLS_PER_CORE + KW - 1  # 526 input cols (with halo)

PR, PC = 8, 16                    # patch = 8 rows x 16 cols = 128 = PE contraction
SC = 33                           # input col-patches per patch-row (528 cols)
SCO = 32                          # output col-patches per patch-row (512 cols)
QI = 514                          # input patch-rows (4096 rows + 16 pad = 4112)
QO = 511                          # output patch-rows (4082 = 510*8 + 2)
NPI = QI * SC                     # 16962 input patches
NPO = QO * SCO                    # 16352 output patches (no halo col)

RT = 16                           # patch-rows per chunk
NT = RT * SCO                     # 512 output patches = one fp32 PSUM bank
NCHUNKS = 32                      # 31 x 512 + 480 (last chunk: 15 patch-rows)
PASSES = [(0, 0), (0, 1), (1, 0), (1, 1), (2, 0), (2, 1)]

# Input slabs: (first_chunk, last_chunk_excl, queue). Emitted up front, before
# any output DMA, so input never queues behind stores. Fine-grained at the
# head so both queues together outrun the PE from chunk 0.
SLAB_PLAN = [
    (0, 1, "sync"),
    (1, 2, "scalar"),
    (2, 4, "sync"),
    (4, 8, "scalar"),
    (8, 16, "sync"),
    (16, 24, "scalar"),
    (24, 32, "sync"),
]
# widest slab: 8 chunks + 2 halo patch-rows (+1: the 3D moving view is sliced
# as nr*SC before dropping the halo column, so it needs one spare patch)
SLABW_MAX = SC * (RT * 8 + 2) + 1

F32 = mybir.dt.float32
BF16 = mybir.dt.bfloat16

# Output DMA plan: (first_chunk, n_chunks, queue). 4-chunk staged groups
# early (scalar + gpsimd), singles alternating sync/scalar at the tail, the
# final chunk split across both HWDGE queues.
OUT_PLAN = [
    (0, 4, "scalar"),
    (4, 4, "gpsimd"),
    (8, 4, "scalar"),
    (12, 4, "gpsimd"),
    (16, 4, "scalar"),
    (20, 2, "sync"),
    (22, 2, "sync"),
    (24, 2, "gpsimd"),
    (26, 2, "scalar"),
    (28, 1, "sync"),
    (29, 1, "scalar"),
    (30, 1, "sync"),
    (31, 1, "split"),
]

N_WARMUP = 24


def _chunk_rows(c):
    """(first_patch_row, n_patch_rows) of chunk c."""
    r0 = RT * c
    return r0, min(RT, QO - r0)


def _build_program():
    nc = bacc.Bacc("TRN2", target_bir_lowering=False, debug=False)
    x = nc.dram_tensor("x", [128, NPI], BF16, kind="ExternalInput").ap()
    wt = nc.dram_tensor("wt", [128, 6 * 128], BF16, kind="ExternalInput").ap()
    out = nc.dram_tensor("out", [128, NPO], F32, kind="ExternalOutput").ap()

    chunk_group = {}
    for g in OUT_PLAN:
        for c in range(g[0], g[0] + g[1]):
            chunk_group[c] = g
    chunk_slab = {}
    for si, (c0, c1, q) in enumerate(SLAB_PLAN):
        for c in range(c0, c1):
            chunk_slab[c] = si

    with tile.TileContext(nc) as tc:
        with (
            tc.tile_pool(name="wpool", bufs=1) as wpool,
            tc.tile_pool(name="xpool", bufs=4) as xpool,
            tc.tile_pool(name="opool", bufs=4) as opool,
            tc.tile_pool(name="dpool", bufs=1) as dpool,
            tc.tile_pool(name="ppool", bufs=7, space="PSUM") as ppool,
            tc.tile_pool(name="dps", bufs=1, space="PSUM") as dps,
        ):
            # All input DMAs up front: weights on scalar, slabs per plan.
            wtile = wpool.tile([128, 6 * 128], BF16, tag="wt")
            nc.scalar.dma_start(wtile[:], wt[:])

            slabs = []
            for c0, c1, q in SLAB_PLAN:
                p_lo = SC * RT * c0
                p_hi = min(SC * (RT * c1 + 2) + 1, NPI)
                st = xpool.tile([128, SLABW_MAX], BF16, tag="xs")
                {"sync": nc.sync, "scalar": nc.scalar}[q].dma_start(
                    st[:, : p_hi - p_lo], x[:, p_lo:p_hi]
                )
                slabs.append((st, p_lo))

            # HAM pre-warm: the PE clock-gate only opens to 2.4GHz after
            # ~3.4us of sustained activity. Dummy matmuls fill the window
            # while slab 0 lands so the real stream starts at full clock.
            dz = dpool.tile([128, 128], F32, tag="dz")
            nc.vector.memset(dz[:], 0)
            dummy = dpool.tile([128, 128], BF16, tag="dummy")
            nc.vector.tensor_copy(dummy[:], dz[:])
            dacc = dps.tile([128, 128], F32)
            for _ in range(N_WARMUP):
                nc.tensor.matmul(dacc[:], dummy[:], dummy[:], start=True, stop=True)

            dma_q = {"sync": nc.sync, "scalar": nc.scalar, "gpsimd": nc.gpsimd}
            stage = None
            for c in range(NCHUNKS):
                st, p_lo = slabs[chunk_slab[c]]
                r0, nr = _chunk_rows(c)
                n = nr * SCO
                acc = ppool.tile([128, NT], F32)
                for i, (dq, ds) in enumerate(PASSES):
                    off = (r0 + dq) * SC + ds - p_lo
                    # moving operand: nr patch-rows of 32 patches, stride 33
                    mv = st[:, off : off + nr * SC].rearrange(
                        "p (r s) -> p r s", s=SC
                    )[:, :, :SCO]
                    nc.tensor.matmul(
                        acc[:, :n],
                        wtile[:, i * 128 : (i + 1) * 128],
                        mv,
                        start=(i == 0),
                        stop=(i == 5),
                    )
                g0, gn_chunks, q = chunk_group[c]
                gi = c - g0
                if gi == 0:
                    stage = opool.tile([128, 4 * NT], F32, tag="ot")
                nc.vector.tensor_copy(stage[:, gi * NT : gi * NT + n], acc[:, :n])
                if gi == gn_chunks - 1:
                    gp0 = NT * g0
                    gn = NT * gi + n
                    if q == "split":
                        h = (gn // 2 + 1) & ~1
                        nc.scalar.dma_start(out[:, gp0 : gp0 + h], stage[:, :h])
                        nc.sync.dma_start(
                            out[:, gp0 + h : gp0 + gn], stage[:, h:gn]
                        )
                    else:
                        dma_q[q].dma_start(out[:, gp0 : gp0 + gn], stage[:, :gn])
    nc.finalize()
    return nc


def _pack_weights(weight: np.ndarray) -> np.ndarray:
    """6 dense 128x128 stationary matrices (one per (dq, ds) pass)."""
    wt = np.zeros((128, 6 * 128), dtype=np.float32)
    rk, ck = np.divmod(np.arange(128)[:, None], PC)  # input pixel within patch
    rm, cm = np.divmod(np.arange(128)[None, :], PC)  # output pixel within patch
    for i, (dq, ds) in enumerate(PASSES):
        a = PR * dq + rk - rm
        b = PC * ds + ck - cm
        valid = (a >= 0) & (a < KH) & (b >= 0) & (b < KW)
        wt[:, i * 128 : (i + 1) * 128] = np.where(
            valid, weight[np.clip(a, 0, KH - 1), np.clip(b, 0, KW - 1)], 0.0
        )
    return wt


def kernel(X: np.ndarray, weight: np.ndarray, bias: np.ndarray) -> np.ndarray:
    X = np.ascontiguousarray(X, dtype=np.float32)
    weight = np.ascontiguousarray(weight, dtype=np.float32)
    bias = np.asarray(bias, dtype=np.float32)

    wt = _pack_weights(weight).astype(ml_dtypes.bfloat16)

    in_maps = []
    for c in range(NCORES):
        xs = np.zeros((QI * PR, SC * PC), dtype=np.float32)
        c0 = c * COLS_PER_CORE
        c1 = min(c0 + IN_COLS, W)
        xs[:H, : c1 - c0] = X[:, c0:c1]
        x_s2d = (
            xs.reshape(QI, PR, SC, PC)
            .transpose(1, 3, 0, 2)
            .reshape(128, NPI)
            .astype(ml_dtypes.bfloat16)
        )
        in_maps.append({"x": np.ascontiguousarray(x_s2d), "wt": wt})

    nc = _build_program()
    res = run_bass_kernel_spmd(nc, in_maps, core_ids=list(range(NCORES)))
    global _last_results
    _last_results = res

    out = np.empty((OH, OW), dtype=np.float32)
    for c in range(NCORES):
        o = np.asarray(res.results[c]["out"], dtype=np.float32)
        o4 = (
            o.reshape(PR, PC, QO, SCO)
            .transpose(2, 0, 3, 1)
            .reshape(QO * PR, SCO * PC)
        )
        c0 = c * COLS_PER_CORE
        n = min(COLS_PER_CORE, OW - c0)
        out[:, c0 : c0 + n] = o4[:OH, :n]

    b0 = float(bias.reshape(-1)[0]) if bias.size else 0.0
    if b0 != 0.0:
        out += b0
    return out
